# revision 24
# baseline (speedup 1.0000x reference)
"""DGCNN point-cloud classifier on 8 Trainium2 NeuronCores.

Sharding: data-parallel over the batch dim B=8 — one point cloud per core.
Each core runs 4 dynamic-kNN edge-conv layers + the 512->1024 linear +
global max/mean pooling locally; the pooled [2048] vectors are AllGathered
and every core computes the (tiny) batch-norm MLP head redundantly.

Edge-conv algebra: h[p,k] = [x_p, x_j - x_p] @ W + b with max over k
  = (x_p @ (Wt - Wb) + b) + max_k (x_j @ Wb)
so only per-point features ever go through matmuls; the kNN gather moves
F-dim rows of c = x @ Wb, done with gpsimd ap_gather in a feature-major
layout.

Top-20 selection per point: the 1024 candidates are split into 16 segments
of 64; DVE max8 gives each segment's top-8 (exact), a 128-wide merge
(max8/match_replace) gives the global top-24 values, and 3 max_index scans
of the full row recover the positions.  This is exact whenever no segment
holds more than 8 of a point's true top-20 (validated: zero occurrences).
"""
import numpy as np

N_CORES = 8
STACK = True
B, P, K, OUT = 8, 1024, 20, 40
T = P // 128  # 8 partition tiles per cloud
EPS = 1e-5
NEG = -1e30

# per-layer (C_in, F_out)
LAYERS = [(3, 64), (64, 64), (64, 128), (128, 256)]

_cache = {}


def _build():
    import concourse.bacc as bacc
    import concourse.mybir as mybir
    from concourse.tile import TileContext

    f32 = mybir.dt.float32
    f32r = mybir.dt.float32r
    u16 = mybir.dt.uint16
    i16 = mybir.dt.int16

    nc = bacc.Bacc(None, num_devices=N_CORES)

    # ---------------- I/O ----------------
    posT = nc.dram_tensor("posT", [3, P], f32, kind="ExternalInput")
    wsub, whalf, bvec = [], [], []
    for li, (C, F) in enumerate(LAYERS):
        wsub.append(nc.dram_tensor(f"wsub{li}", [C, F], f32, kind="ExternalInput"))
        whalf.append(nc.dram_tensor(f"whalf{li}", [C, F], f32, kind="ExternalInput"))
        bvec.append(nc.dram_tensor(f"bvec{li}", [F, 1], f32, kind="ExternalInput"))
    wm = nc.dram_tensor("wm", [512, 1024], f32, kind="ExternalInput")
    bm = nc.dram_tensor("bm", [1024, 1], f32, kind="ExternalInput")
    wa = nc.dram_tensor("wa", [2048, 512], f32, kind="ExternalInput")
    ba = nc.dram_tensor("ba", [512, 1], f32, kind="ExternalInput")
    ga = nc.dram_tensor("ga", [512, 1], f32, kind="ExternalInput")
    bea = nc.dram_tensor("bea", [512, 1], f32, kind="ExternalInput")
    wbh = nc.dram_tensor("wbh", [512, 256], f32, kind="ExternalInput")
    bbh = nc.dram_tensor("bbh", [256, 1], f32, kind="ExternalInput")
    gb = nc.dram_tensor("gb", [256, 1], f32, kind="ExternalInput")
    beb = nc.dram_tensor("beb", [256, 1], f32, kind="ExternalInput")
    wc = nc.dram_tensor("wc", [256, 40], f32, kind="ExternalInput")
    bc = nc.dram_tensor("bc", [40, 1], f32, kind="ExternalInput")
    y_out = nc.dram_tensor("y", [B, OUT], f32, kind="ExternalOutput")

    cc_in = nc.dram_tensor("cc_in", [1, 2048], f32, kind="Internal")
    cc_out = nc.dram_tensor("cc_out", [B, 2048], f32, kind="Internal",
                            addr_space="Shared")

    AG = mybir.AxisListType
    ALU = mybir.AluOpType
    ACTF = mybir.ActivationFunctionType

    with TileContext(nc) as tc:
        with tc.tile_pool(name="const", bufs=1) as cpool, \
             tc.tile_pool(name="dram", bufs=3, space="DRAM") as dpool:
            # ---------------- resident SBUF tensors ----------------
            ONES = cpool.tile([1, P], f32)
            nc.vector.memset(ONES[:], 1.0)
            NEGCOL = cpool.tile([128, 1], f32)
            nc.vector.memset(NEGCOL[:], -1.0)
            EPSC = cpool.tile([128, 1], f32)
            nc.vector.memset(EPSC[:], EPS)

            # feature buffers (x^T per layer)
            L1 = cpool.tile([4, P], f32)
            L2 = cpool.tile([65, P], f32)
            L3 = cpool.tile([65, P], f32)
            L4 = cpool.tile([128, P], f32)
            X4a = cpool.tile([128, P], f32)
            X4b = cpool.tile([128, P], f32)
            Lbufs = [L1, L2, L3, L4]

            R1 = cpool.tile([4, P], f32)
            R2 = cpool.tile([65, P], f32)
            R3 = cpool.tile([65, P], f32)
            R4 = cpool.tile([128, P], f32)
            NSQ4 = cpool.tile([1, P], f32)
            NSQ1 = cpool.tile([1, P], f32)
            Rbufs = [R1, R2, R3, R4]

            # a/c tables, double-buffered by layer parity so the next
            # layer's tables can be built while this layer's gathers run.
            # For layers with F=64 the c features are replicated into
            # partitions 64:128 so a 2-tile stacked gather/reduce can be used
            ATa1 = cpool.tile([128, P], f32)
            ATa2 = cpool.tile([128, P], f32)
            ATb1 = cpool.tile([128, P], f32)
            ATb2 = cpool.tile([128, P], f32)
            CTa1 = cpool.tile([128, P], f32)
            CTa2 = cpool.tile([128, P], f32)
            CTb1 = cpool.tile([128, P], f32)
            CTb2 = cpool.tile([128, P], f32)
            ATs_par = [[ATa1, ATa2], [ATb1, ATb2]]
            CTs_par = [[CTa1, CTa2], [CTb1, CTb2]]
            IDE8 = cpool.tile([8, 8], f32)
            IOT8 = cpool.tile([8, 8], mybir.dt.int32)
            nc.gpsimd.iota(IOT8[:], pattern=[[1, 8]], base=0,
                           channel_multiplier=-1)
            nc.vector.tensor_scalar(IDE8[:], IOT8[:], 0, None,
                                    op0=ALU.is_equal)

            # weights in SBUF
            ws_sb, wh_sb, b_sb = [], [], []
            for li, (C, F) in enumerate(LAYERS):
                w1 = cpool.tile([C, F], f32, tag=f"ws{li}")
                w2 = cpool.tile([C, F], f32, tag=f"wh{li}")
                bb_ = cpool.tile([min(F, 128), (F + 127) // 128], f32, tag=f"bv{li}")
                nc.sync.dma_start(w1[:], wsub[li][:])
                nc.sync.dma_start(w2[:], whalf[li][:])
                for mt in range((F + 127) // 128):
                    r0, r1 = 128 * mt, min(F, 128 * (mt + 1))
                    nc.sync.dma_start(bb_[0:r1 - r0, mt:mt + 1], bvec[li][r0:r1, :])
                ws_sb.append(w1)
                wh_sb.append(w2)
                b_sb.append(bb_)

            # Wm K-tiles; DMAs deferred until after layer 1 starts
            wm_rows = [(0, 64), (64, 128), (128, 256), (256, 384), (384, 512)]
            wm_sb = []
            for i, (r0, r1) in enumerate(wm_rows):
                t_ = cpool.tile([r1 - r0, 1024], f32r, tag=f"wm{i}")
                wm_sb.append(t_)
            bm_sb = cpool.tile([128, 8], f32)
            wa_sb, wbh_sb, wc_sb = [], [], []
            for k in range(16):
                wa_t = cpool.tile([128, 512], f32, tag=f"wa{k}")
                wa_sb.append(wa_t)
            for k in range(4):
                wbh_t = cpool.tile([128, 256], f32, tag=f"wbh{k}")
                wbh_sb.append(wbh_t)
            for k in range(2):
                wc_t = cpool.tile([128, 40], f32, tag=f"wc{k}")
                wc_sb.append(wc_t)
            ba_sb = cpool.tile([128, 4], f32, tag="ba")
            ga_sb = cpool.tile([128, 4], f32, tag="ga")
            bea_sb = cpool.tile([128, 4], f32, tag="bea")
            bbh_sb = cpool.tile([128, 2], f32, tag="bbh")
            gb_sb = cpool.tile([128, 2], f32, tag="gb")
            beb_sb = cpool.tile([128, 2], f32, tag="beb")
            bc_sb = cpool.tile([40, 1], f32)

            def load_big_weights():
                for t_, (r0, r1) in zip(wm_sb, wm_rows):
                    nc.sync.dma_start(t_[:], wm[r0:r1, :].bitcast(f32r))
                for mt in range(8):
                    nc.sync.dma_start(bm_sb[:, mt:mt + 1],
                                      bm[128 * mt:128 * (mt + 1), :])
                for k in range(16):
                    nc.sync.dma_start(wa_sb[k][:], wa[128 * k:128 * (k + 1), :])
                for k in range(4):
                    nc.sync.dma_start(wbh_sb[k][:], wbh[128 * k:128 * (k + 1), :])
                for k in range(2):
                    nc.sync.dma_start(wc_sb[k][:], wc[128 * k:128 * (k + 1), :])
                for t_, dram, blocks in ((ba_sb, ba, 4), (ga_sb, ga, 4),
                                         (bea_sb, bea, 4), (bbh_sb, bbh, 2),
                                         (gb_sb, gb, 2), (beb_sb, beb, 2)):
                    for mt in range(blocks):
                        nc.sync.dma_start(t_[:, mt:mt + 1],
                                          dram[128 * mt:128 * (mt + 1), :])
                nc.sync.dma_start(bc_sb[:], bc[:])

            nc.sync.dma_start(L1[0:3, :], posT[:])
            nc.sync.dma_start(L1[3:4, :], ONES[:])
            nc.sync.dma_start(L2[64:65, :], ONES[:])
            nc.sync.dma_start(L3[64:65, :], ONES[:])

            with tc.tile_pool(name="ps", bufs=2, space="PSUM") as pspool, \
                 tc.tile_pool(name="ps2", bufs=2, space="PSUM") as ps2pool, \
                 tc.tile_pool(name="work", bufs=2) as wpool, \
                 tc.tile_pool(name="sd", bufs=2) as sdpool, \
                 tc.tile_pool(name="one", bufs=1) as opool, \
                 tc.tile_pool(name="gathp", bufs=2) as gpool, \
                 tc.tile_pool(name="idxp", bufs=3) as ipool:

                GP = cpool.tile([128, 16], f32)

                cat_srcs = [(L2, 64), (L3, 64), (L4, 128), (X4a, 128),
                            (X4b, 128)]
                catr = []
                for i, (buf, rr) in enumerate(cat_srcs):
                    cr_t = cpool.tile([rr, P], f32r, tag=f"catr{i}")
                    catr.append(cr_t)

                def round_cat_half(n):
                    for (buf, rr), cr_ in zip(cat_srcs, catr):
                        nc.scalar.activation(cr_[:, 512 * n:512 * (n + 1)],
                                             buf[0:rr, 512 * n:512 * (n + 1)],
                                             ACTF.Copy)

                def do_wm_half(n):
                    # cat @ Wm for column half n (512 points) + pooling partials
                    for mt in range(8):
                        mc0, mc1 = 128 * mt, 128 * (mt + 1)
                        ops = pspool.tile([128, 512], f32, tag="wmh")
                        for kt, (cr_, wmk) in enumerate(zip(catr, wm_sb)):
                            nc.tensor.matmul(
                                ops[:],
                                wmk[:, mc0:mc1],
                                cr_[:, 512 * n:512 * (n + 1)],
                                start=(kt == 0), stop=(kt == 4))
                        gmx = wpool.tile([128, 1], f32, tag="gmx")
                        nc.vector.tensor_reduce(gmx[:], ops[:], axis=AG.X,
                                                op=ALU.max)
                        osb = opool.tile([128, 512], f32, tag="osb")
                        gsm = wpool.tile([128, 1], f32, tag="gsm")
                        nc.scalar.activation(osb[:], ops[:], ACTF.Copy,
                                             accum_out=gsm[:])
                        if n == 0:
                            nc.scalar.activation(GP[:, mt:mt + 1], gmx[:], ACTF.Copy)
                            nc.scalar.activation(GP[:, 8 + mt:9 + mt], gsm[:],
                                                 ACTF.Copy)
                        else:
                            nc.vector.tensor_tensor(GP[:, mt:mt + 1],
                                                    GP[:, mt:mt + 1], gmx[:],
                                                    op=ALU.max)
                            nc.vector.tensor_tensor(GP[:, 8 + mt:9 + mt],
                                                    GP[:, 8 + mt:9 + mt], gsm[:],
                                                    op=ALU.add)

                def prep_half(li, n):
                    # R rows = 2*x^T, sqx, negsq row — for column half n
                    C, F = LAYERS[li]
                    Lb, Rb = Lbufs[li], Rbufs[li]
                    h0, h1 = 512 * n, 512 * (n + 1)
                    with tc.high_priority(offset=-30):
                        nc.scalar.activation(Rb[0:C, h0:h1], Lb[0:C, h0:h1],
                                             ACTF.Copy, scale=2.0)
                        sqx = opool.tile([128, 512], f32, tag="sqx")
                        nc.scalar.activation(sqx[0:C, :], Lb[0:C, h0:h1],
                                             ACTF.Square)
                        nps = ps2pool.tile([128, 512], f32, tag="pre", bufs=3)
                        if li in (1, 2):
                            # write negsq directly into partition C=64
                            nc.tensor.matmul(nps[64:65, :],
                                             NEGCOL[0:C, :], sqx[0:C, :],
                                             start=True, stop=True)
                            nc.scalar.activation(Rb[64:65, h0:h1],
                                                 nps[64:65, :], ACTF.Copy)
                        else:
                            nc.tensor.matmul(nps[0:1, :],
                                             NEGCOL[0:C, :], sqx[0:C, :],
                                             start=True, stop=True)
                            dstn = NSQ1 if li == 0 else NSQ4
                            nc.scalar.activation(dstn[:, h0:h1],
                                                 nps[0:1, :], ACTF.Copy)
                            if li == 0:
                                nc.sync.dma_start(Rb[3:4, h0:h1],
                                                  dstn[:, h0:h1])

                def ac_half(li, n):
                    # a^T = wsub^T x + b ;  c^T = whalf^T x  for column half n
                    C, F = LAYERS[li]
                    Lb = Lbufs[li]
                    n_mt = (F + 127) // 128
                    stacked = STACK and F == 64
                    ATs = ATs_par[li % 2]
                    CTs = CTs_par[li % 2]
                    h0, h1 = 512 * n, 512 * (n + 1)
                    with tc.high_priority(offset=-60):
                        for mt in range(n_mt):
                            Fm = min(128, F - 128 * mt)
                            aps = ps2pool.tile([128, 512], f32, tag="pre", bufs=3)
                            cps = ps2pool.tile([128, 512], f32, tag="pre", bufs=3)
                            nc.tensor.matmul(
                                aps[0:Fm, :],
                                ws_sb[li][:, 128 * mt:128 * mt + Fm],
                                Lb[0:C, h0:h1],
                                start=True, stop=True)
                            nc.tensor.matmul(
                                cps[0:Fm, :],
                                wh_sb[li][:, 128 * mt:128 * mt + Fm],
                                Lb[0:C, h0:h1],
                                start=True, stop=True)
                            if stacked:
                                # replicate c into partitions 64:128 for the
                                # stacked 2-tile gather
                                nc.tensor.matmul(
                                    cps[64:64 + Fm, :],
                                    wh_sb[li][:, 128 * mt:128 * mt + Fm],
                                    Lb[0:C, h0:h1],
                                    start=True, stop=True)
                            nc.scalar.activation(ATs[mt][0:Fm, h0:h1],
                                                 aps[0:Fm, :], ACTF.Identity,
                                                 bias=b_sb[li][0:Fm, mt:mt + 1])
                            Fc = 64 + Fm if stacked else Fm
                            nc.scalar.activation(CTs[mt][0:Fc, h0:h1],
                                                 cps[0:Fc, :], ACTF.Copy)

                pending = []

                def flush_pending():
                    with tc.high_priority(offset=-60):
                        for (kind, args) in pending:
                            if kind == "tr":
                                g_, Fm_, dst_, AT_, a0, a1 = args
                                red = wpool.tile([128, 128], f32, tag="red")
                                nc.vector.tensor_reduce(
                                    red[0:Fm_, :],
                                    g_[0:Fm_, :].rearrange(
                                        "c (p k) -> c p k", k=K),
                                    axis=AG.X, op=ALU.max)
                                nc.gpsimd.tensor_add(dst_[0:Fm_, a0:a1],
                                                     red[0:Fm_, :],
                                                     AT_[0:Fm_, a0:a1])
                            else:  # "tr2"
                                # stacked pair: one reduce [128,2560] covers
                                # two tiles; tile B's rows move cross-partition
                                # via a small DMA
                                g_, dst_, AT_, a0, b0 = args
                                red = wpool.tile([128, 128], f32, tag="red")
                                nc.vector.tensor_reduce(
                                    red[:],
                                    g_[:].rearrange("c (p k) -> c p k", k=K),
                                    axis=AG.X, op=ALU.max)
                                nc.gpsimd.tensor_add(dst_[0:64, a0:a0 + 128],
                                                     red[0:64, :],
                                                     AT_[0:64, a0:a0 + 128])
                                redB = wpool.tile([64, 128], f32, tag="redB")
                                nc.sync.dma_start(redB[:], red[64:128, :])
                                nc.gpsimd.tensor_add(dst_[0:64, b0:b0 + 128],
                                                     redB[:],
                                                     AT_[0:64, b0:b0 + 128])
                    pending.clear()

                early = {}

                def topk_half(li, t, sd, V, n):
                    C, F = LAYERS[li]
                    Lb, Rb = Lbufs[li], Rbufs[li]
                    tc0, tc1 = 128 * t, 128 * (t + 1)
                    sps = pspool.tile([128, 512], f32, tag="s", bufs=3)
                    if li < 3:
                        nc.tensor.matmul(
                            sps[:],
                            Lb[0:C + 1, tc0:tc1],
                            Rb[0:C + 1, 512 * n:512 * (n + 1)],
                            start=True, stop=True)
                    else:
                        nc.tensor.matmul(
                            sps[:],
                            Lb[0:C, tc0:tc1],
                            Rb[0:C, 512 * n:512 * (n + 1)],
                            start=True, stop=False)
                        nc.tensor.matmul(
                            sps[:],
                            ONES[:, tc0:tc1],
                            NSQ4[:, 512 * n:512 * (n + 1)],
                            start=False, stop=True)
                    nc.scalar.activation(sd[:, 512 * n:512 * (n + 1)],
                                         sps[:], ACTF.Copy)
                    for g in range(8):
                        c0 = 512 * n + 64 * g
                        nc.vector.max(
                            V[:, 64 * n + 8 * g:64 * n + 8 * g + 8],
                            sd[:, c0:c0 + 64])

                def early_half0(li, tiles):
                    # emit first-half gram/drain/segment-max for the next
                    # layer's low tiles while this layer still runs
                    for t in tiles:
                        sd = sdpool.tile([128, P], f32, tag="sd", bufs=2)
                        V = ipool.tile([128, 128], f32, tag="V", bufs=2)
                        topk_half(li, t, sd, V, 0)
                        early[(li, t)] = (sd, V)

                def topk_tile(li, t):
                    # gram for tile t; returns idxc [128,24] u16
                    if (li, t) in early:
                        sd, V = early.pop((li, t))
                    else:
                        sd = sdpool.tile([128, P], f32, tag="sd", bufs=2)
                        V = ipool.tile([128, 128], f32, tag="V", bufs=2)
                        topk_half(li, t, sd, V, 0)
                    topk_half(li, t, sd, V, 1)
                    vv = ipool.tile([128, 24], f32, tag="vv")
                    idxc = ipool.tile([128, 24], u16, tag="idxc")
                    nc.vector.max(vv[:, 0:8], V[:])
                    nc.vector.match_replace(V[:], vv[:, 0:8], V[:], NEG)
                    nc.vector.max(vv[:, 8:16], V[:])
                    nc.vector.match_replace(V[:], vv[:, 8:16], V[:], NEG)
                    nc.vector.max(vv[:, 16:24], V[:])
                    nc.vector.max_index(idxc[:, 0:8], vv[:, 0:8], sd[:])
                    nc.vector.max_index(idxc[:, 8:16], vv[:, 8:16], sd[:])
                    nc.vector.max_index(idxc[:, 16:24], vv[:, 16:24], sd[:])
                    return idxc

                def idx_to_groups(idxc, idxw, part0, ngroups):
                    # 4-DMA chain: flat p-major -> group-wrap transpose ->
                    # replicate in DRAM -> one contiguous load into the
                    # target partitions.  Total ~1.5us vs 9us for per-group
                    # transposed loads.
                    d1 = dpool.tile([160, 16], i16, tag="d1")
                    nc.sync.dma_start(d1[:].bitcast(u16), idxc[:, 0:20])
                    d2 = dpool.tile([16, 160], i16, tag="d2")
                    nc.sync.dma_start(d2[:], d1[:].rearrange("j w -> w j"))
                    d3 = dpool.tile([16 * ngroups, 160], i16, tag="d3")
                    nc.sync.dma_start(
                        d3[:].rearrange("(r w) j -> r w j", w=16),
                        d2[:].partition_broadcast(ngroups))
                    nc.sync.dma_start(idxw[part0:part0 + 16 * ngroups, :],
                                      d3[:])

                prep_half(0, 0)
                prep_half(0, 1)
                ac_half(0, 0)
                ac_half(0, 1)

                for li, (C, F) in enumerate(LAYERS):
                    if li == 1:
                        load_big_weights()
                    n_mt = (F + 127) // 128
                    stacked = STACK and F == 64
                    ATs = ATs_par[li % 2]
                    CTs = CTs_par[li % 2]
                    if li < 3:
                        outs_mt = [Lbufs[li + 1]]
                    else:
                        outs_mt = [X4a, X4b]

                    if stacked:
                        # two tiles per gather instruction (per-group indices)
                        for tp in range(T // 2):
                            tA, tB = 2 * tp, 2 * tp + 1
                            idxA = topk_tile(li, tA)
                            idxw = ipool.tile([128, 160], i16, tag="idxw", bufs=2)
                            idx_to_groups(idxA, idxw, 0, 4)
                            idxB = topk_tile(li, tB)
                            idx_to_groups(idxB, idxw, 64, 4)
                            flush_pending()
                            if tp == 2 and li < 3:
                                # first half of this layer's output is ready:
                                # emit the next layer's first-half prep and
                                # the low tiles' first-half top-k work as
                                # gap fillers (far lower priority)
                                with tc.high_priority(offset=-60):
                                    prep_half(li + 1, 0)
                                    ac_half(li + 1, 0)
                            gath = gpool.tile([128, K * 128], f32, tag="gath")
                            nc.gpsimd.ap_gather(
                                gath[:],
                                CTs[0][:].rearrange("c (n d) -> c n d", d=1),
                                idxw[:],
                                channels=128, num_elems=P, d=1, num_idxs=K * 128)
                            pending.append(("tr2", (gath, outs_mt[0], ATs[0],
                                                    128 * tA, 128 * tB)))
                    else:
                        for t in range(T):
                            idxc = topk_tile(li, t)
                            idxw = ipool.tile([128, 160], i16, tag="idxw", bufs=2)
                            idx_to_groups(idxc, idxw, 0, 8)
                            flush_pending()
                            if t == 4 and li == 2:
                                with tc.high_priority(offset=-60):
                                    prep_half(3, 0)
                                    ac_half(3, 0)
                            for mt in range(n_mt):
                                Fm = min(128, F - 128 * mt)
                                Fg = ((Fm + 15) // 16) * 16
                                gath = gpool.tile([128, K * 128], f32, tag="gath")
                                nc.gpsimd.ap_gather(
                                    gath[0:Fg, :],
                                    CTs[mt][0:Fg, :].rearrange(
                                        "c (n d) -> c n d", d=1),
                                    idxw[0:Fg, :],
                                    channels=Fg, num_elems=P, d=1,
                                    num_idxs=K * 128)
                                dst = outs_mt[mt] if li == 3 else outs_mt[0]
                                pending.append(("tr", (gath, Fm, dst, ATs[mt],
                                                       128 * t, 128 * (t + 1))))
                            # start the Wm half over points 0:512 as soon as
                            # the first half of layer 4 is aggregated
                            if li == 3 and t == 4:
                                flush_pending()
                                with tc.high_priority(offset=-70):
                                    round_cat_half(0)
                                    do_wm_half(0)

                    flush_pending()
                    if li < 3:
                        prep_half(li + 1, 1)
                        ac_half(li + 1, 1)

                # ---------------- finish cat @ Wm + pooling ----------------
                with tc.high_priority(offset=-70):
                    round_cat_half(1)
                    do_wm_half(1)
                    dumq = wpool.tile([128, 1], f32, tag="mu")
                    nc.scalar.activation(dumq[:], EPSC[:], ACTF.Sqrt)

                # gmax += bm ; gmean = gmean/P + bm
                nc.vector.tensor_tensor(GP[:, 0:8], GP[:, 0:8], bm_sb[:], op=ALU.add)
                nc.vector.tensor_scalar(GP[:, 8:16], GP[:, 8:16], 1.0 / P, None,
                                        op0=ALU.mult)
                nc.vector.tensor_tensor(GP[:, 8:16], GP[:, 8:16], bm_sb[:], op=ALU.add)

                # pooled [2048] -> cc_in, AllGather
                nc.sync.dma_start(
                    cc_in[:].rearrange("o (m f) -> o f m", f=128),
                    GP[:, 0:16])
                nc.gpsimd.collective_compute(
                    "AllGather", ALU.bypass,
                    replica_groups=[list(range(N_CORES))],
                    ins=[cc_in[:].opt()], outs=[cc_out[:].opt()])

                # ---------------- head (redundant on every core) ----------------
                HT = cpool.tile([128, 128], f32)   # H^T K-tiles: col block k = [128,8]
                nc.sync.dma_start(X4a[0:8, :], cc_out[:, 0:1024])
                nc.sync.dma_start(X4b[0:8, :], cc_out[:, 1024:2048])
                tps = ps2pool.tile([128, 128], f32, tag="pre", bufs=3)
                for k in range(16):
                    hsrc = X4a if k < 8 else X4b
                    nc.tensor.transpose(
                        tps[:, 8 * k:8 * (k + 1)],
                        hsrc[0:8, 128 * (k % 8):128 * (k % 8 + 1)], IDE8[:])
                nc.scalar.activation(HT[:], tps[:], ACTF.Copy)

                def bn_leaky(src, blocks, gamma, beta):
                    # src [128, 8*blocks]; batch-norm over free dim (batch)
                    for mt in range(blocks):
                        blk = src[:, 8 * mt:8 * (mt + 1)]
                        mu = wpool.tile([128, 1], f32, tag="mu")
                        nc.vector.tensor_reduce(mu[:], blk, axis=AG.X, op=ALU.add)
                        nc.vector.tensor_scalar(mu[:], mu[:], 1.0 / 8, None,
                                                op0=ALU.mult)
                        nc.vector.tensor_scalar(blk, blk, mu[:], None,
                                                op0=ALU.subtract)
                        sq2 = wpool.tile([128, 8], f32, tag="sq2")
                        nc.scalar.activation(sq2[:], blk, ACTF.Square)
                        var = wpool.tile([128, 1], f32, tag="var")
                        nc.vector.tensor_reduce(var[:], sq2[:], axis=AG.X,
                                                op=ALU.add)
                        nc.scalar.activation(var[:], var[:], ACTF.Sqrt,
                                             scale=1.0 / 8, bias=EPSC[:])
                        nc.vector.reciprocal(var[:], var[:])
                        nc.vector.tensor_scalar(blk, blk, var[:], None,
                                                op0=ALU.mult)
                        nc.vector.tensor_scalar(blk, blk, gamma[:, mt:mt + 1],
                                                beta[:, mt:mt + 1], op0=ALU.mult,
                                                op1=ALU.add)
                        lk = wpool.tile([128, 8], f32, tag="lk")
                        nc.vector.tensor_scalar(lk[:], blk, 0.2, None, op0=ALU.mult)
                        nc.vector.tensor_tensor(blk, blk, lk[:], op=ALU.max)

                HA = cpool.tile([128, 32], f32)
                for mt in range(4):
                    hps = pspool.tile([128, 8], f32, tag="wmh")
                    for k in range(16):
                        nc.tensor.matmul(hps[:], wa_sb[k][:, 128 * mt:128 * (mt + 1)],
                                         HT[:, 8 * k:8 * (k + 1)],
                                         start=(k == 0), stop=(k == 15))
                    nc.scalar.activation(HA[:, 8 * mt:8 * (mt + 1)], hps[:],
                                         ACTF.Identity, bias=ba_sb[:, mt:mt + 1])
                bn_leaky(HA, 4, ga_sb, bea_sb)

                HB = cpool.tile([128, 16], f32)
                for mt in range(2):
                    hps = pspool.tile([128, 8], f32, tag="wmh")
                    for k in range(4):
                        nc.tensor.matmul(hps[:], wbh_sb[k][:, 128 * mt:128 * (mt + 1)],
                                         HA[:, 8 * k:8 * (k + 1)],
                                         start=(k == 0), stop=(k == 3))
                    nc.scalar.activation(HB[:, 8 * mt:8 * (mt + 1)], hps[:],
                                         ACTF.Identity, bias=bbh_sb[:, mt:mt + 1])
                bn_leaky(HB, 2, gb_sb, beb_sb)

                ops2 = pspool.tile([128, 8], f32, tag="wmh")
                for k in range(2):
                    nc.tensor.matmul(ops2[0:40, :], wc_sb[k][:, :],
                                     HB[:, 8 * k:8 * (k + 1)],
                                     start=(k == 0), stop=(k == 1))
                outs = cpool.tile([40, 8], f32)
                nc.scalar.activation(outs[:], ops2[0:40, :], ACTF.Identity, bias=bc_sb[:])
                nc.sync.dma_start(y_out[:].rearrange("b f -> f b"), outs[:])

    nc.finalize()
    return nc


def _prep_inputs(inputs):
    """Host-side sharding + weight reparametrization; all fp32."""
    f = np.float32
    pos = np.ascontiguousarray(inputs["pos"], dtype=f).reshape(B, P, 3)
    maps = []
    names = [("W1", "b1"), ("W2", "b2"), ("W3", "b3"), ("W4", "b4")]
    common = {}
    for li, (C, F) in enumerate(LAYERS):
        W = np.asarray(inputs[names[li][0]], dtype=f)
        b = np.asarray(inputs[names[li][1]], dtype=f)
        common[f"wsub{li}"] = np.ascontiguousarray(W[:C] - W[C:])
        common[f"whalf{li}"] = np.ascontiguousarray(W[C:])
        common[f"bvec{li}"] = b.reshape(F, 1)
    common["wm"] = np.asarray(inputs["Wm"], dtype=f)
    common["bm"] = np.asarray(inputs["bm"], dtype=f).reshape(1024, 1)
    common["wa"] = np.asarray(inputs["Wa"], dtype=f)
    common["ba"] = np.asarray(inputs["ba"], dtype=f).reshape(512, 1)
    common["ga"] = np.asarray(inputs["ga"], dtype=f).reshape(512, 1)
    common["bea"] = np.asarray(inputs["bea"], dtype=f).reshape(512, 1)
    common["wbh"] = np.asarray(inputs["Wb"], dtype=f)
    common["bbh"] = np.asarray(inputs["bb"], dtype=f).reshape(256, 1)
    common["gb"] = np.asarray(inputs["gb"], dtype=f).reshape(256, 1)
    common["beb"] = np.asarray(inputs["beb"], dtype=f).reshape(256, 1)
    common["wc"] = np.asarray(inputs["Wc"], dtype=f)
    common["bc"] = np.asarray(inputs["bc"], dtype=f).reshape(40, 1)
    for c in range(N_CORES):
        m = dict(common)
        m["posT"] = np.ascontiguousarray(pos[c].T)
        maps.append(m)
    return maps


def kernel(**inputs) -> np.ndarray:
    from concourse.bass_utils import run_bass_kernel_spmd

    if "nc" not in _cache:
        _cache["nc"] = _build()
    nc = _cache["nc"]
    in_maps = _prep_inputs(inputs)
    res = run_bass_kernel_spmd(nc, in_maps, core_ids=list(range(N_CORES)))
    return np.asarray(res.results[0]["y"], dtype=np.float32)
